# revision 38
# baseline (speedup 1.0000x reference)
"""BitLinear (BitNet b1.58) TP kernel for 8 NeuronCores.

y = fake_quant_act(x) @ ternary_absmean(W).T + bias

Sharding: W (and bias) split along out_features across 8 cores; x replicated.
Per core: W shard [1024, 8192] f32, x [16, 8192] f32 -> y shard [16, 1024].

v5 design (HW-measured ~119 us vs ~135 us baseline; DMA floor ~102 us):
  - SP DGE ring carries x first, then ONLY the wb weight stream, plus
    tiny y stores one/two blocks late (their data is long ready) so the
    stream never head-of-line stalls.  Constants ride the ACT ring.
  - Matmul orientation produces y TRANSPOSED: psgT[o,b] += gbuf[k,o].T
    @ xiT[k,b].  alpha / bias are then per-PARTITION, so the epilogue
    is two tiny DVE ops: ysbT = alpha_sc*(psgT + corr) + bias_col,
    where corr = -192*sum_k xi[b,k] (outer ones_o, removes the +192
    quant offset) is ONE k=1 f32 matmul into its own PSUM bank per
    iteration -- kept out of the per-block bf16 accumulation groups
    (mixed-dtype groups showed a run-to-run precision wobble on HW).
    The epilogue is emitted one block late so the in-order engines
    never wait on the PE stop.  y is stored [o,b], transposed on host.
  - Per block (128 rows x 8192 cols), work flows in 8 column-groups:
    DVE quant group (bf16(rinv*w+192), RNE == round) -> PE 8 transposes
    -> PSUM->SBUF copy -> PE 8 matmuls (2-group lag).  Copies: groups
    0,2 via ACT (f32-bitcast copy of a DVE-pre-clamped source; 4x-rate
    bf16 pass), rest via DVE with the clamp to {191,192,193} fused.
    NO gpsimd in the flow: Pool cannot touch PSUM, and its SBUF ops
    measured far slower on HW than the cost model (+12 us total).
  - abs row-sums (for alpha) on ACT in halves, emitted adjacent so ACT
    chases the DMA lands; the LAST block's second half lands as 4
    quarter-DMAs chased by 3 ACT pieces + a DVE tail piece, and its
    quant is split DVE(g0-g4)/ACT(g5-g7, activation Identity with
    per-row scale) to shorten the serial tail.
  - Measured-slower on HW (despite equal cost-model time): Pool
    pre-clamps/copies/epilogue, 4-way tail copy split + split tail h0
    (138 us!), NG=16 groups, x on the ACT ring.
"""

import contextlib
import os
import numpy as np

import concourse.bass as bass
import concourse.bacc as bacc
import concourse.mybir as mybir
from concourse import tile
from concourse.bass_utils import run_bass_kernel_spmd

F32 = mybir.dt.float32
BF16 = mybir.dt.bfloat16

B = 16          # batch
K = 8192        # in_features
OUT = 8192      # out_features
NCORES = 8
OSH = OUT // NCORES      # 1024 out rows per core
P = 128
NBLK = OSH // P          # 8 o-blocks per core
KT = K // P              # 64 k-tiles
NG = 8                   # col groups per block (8 k-tiles = 1024 cols each)
GW = K // NG             # 1024 cols per group
EPS = 1e-8
QMAX = 127.0
MAGIC32 = 1.5 * 2.0**23  # f32 round-to-nearest-even magic
OFF = 192.0              # bf16 round magic (ulp=1 in [128,256))

HALF = K // 2
QTR = K // 4

_CACHE = {}
PRECLAMP_ENG = "dve"   # "pool" to offload mid-stream pre-clamps to gpsimd


def _copy_eng(g, tail=False):
    # PSUM->SBUF copy engine per group: ACT (bitcast f32, pre-clamped
    # source) for even-ish groups, DVE (clamp fused) otherwise
    if tail:
        return "act" if g in (0, 2) else "dve"
    return "act" if g in (0, 2) else "dve"


def _build_bass(loop_r=None, py_r=1, skip=()):
    nc = bacc.Bacc()
    x_d = nc.declare_dram_parameter("x", [B, K], F32, isOutput=False)
    w_d = nc.declare_dram_parameter("w", [OSH, K], F32, isOutput=False)
    bc_d = nc.declare_dram_parameter("biasc", [P, NBLK], F32, isOutput=False)
    id_d = nc.declare_dram_parameter("ident", [P, P], F32, isOutput=False)
    idb_d = nc.declare_dram_parameter("identb", [P, P], BF16, isOutput=False)
    y_d = nc.declare_dram_parameter("y", [OSH, B], F32, isOutput=True)

    with tile.TileContext(nc) as tc:
        with (
            tc.tile_pool(name="const", bufs=1) as cpool,
            tc.tile_pool(name="wp", bufs=3) as wpool,
            tc.tile_pool(name="qp", bufs=3) as qpool,
            tc.tile_pool(name="gp", bufs=2) as gpool,
            tc.tile_pool(name="sm", bufs=2) as smpool,
            tc.tile_pool(name="xp", bufs=2) as xpool,
            tc.tile_pool(name="al", bufs=4) as apool,
            tc.tile_pool(name="ps_y", bufs=2, space="PSUM") as ps_y,
            tc.tile_pool(name="ps_t", bufs=3, space="PSUM") as ps_t,
            tc.tile_pool(name="ps_c", bufs=1, space="PSUM") as ps_c,
            tc.tile_pool(name="ps_ar", bufs=1, space="PSUM") as ps_ar,
            tc.tile_pool(name="ps_sb", bufs=1, space="PSUM") as ps_sb,
        ):
            # ---- constants (one-time, ACT ring) ----
            ident = cpool.tile([P, P], F32, tag="ident")
            nc.scalar.dma_start(ident[:], id_d[:])
            identb = cpool.tile([P, P], BF16, tag="identb")
            nc.scalar.dma_start(identb[:], idb_d[:])
            ones_row = cpool.tile([1, P], F32, tag="ones_row")
            nc.vector.memset(ones_row[:], 1.0)
            off192 = cpool.tile([P, 1], F32, tag="off192")
            nc.vector.memset(off192[:], OFF)
            biasc = cpool.tile([P, NBLK], F32, tag="biasc")
            nc.scalar.dma_start(biasc[:], bc_d[:])

            loop = tc.For_i(0, loop_r) if loop_r else contextlib.nullcontext()
            for _py_rep in range(py_r):
              with loop:
                # ---- weight stream (SP ring) ----
                def emit_load(i, tail=False):
                    wb = wpool.tile([P, K], F32, tag="wb")
                    cuts = [0, HALF, K] if not tail else \
                        [0, HALF, HALF + QTR // 2, 3 * QTR,
                         3 * QTR + QTR // 2, K]
                    for lo, hi in zip(cuts[:-1], cuts[1:]):
                        nc.sync.dma_start(
                            wb[:, lo:hi],
                            w_d[i * P:(i + 1) * P, lo:hi],
                        )
                    return wb

                # x rides the SP ring FIRST (tiny, no deps, and the whole
                # x-prep chain sits ahead of block 0's alpha in DVE
                # program order -- a late x would stall every block).
                # load x as [128, 1024]: partition = (b*8+g), free = f,
                # k = g*1024+f
                x128 = cpool.tile([P, K // 8], F32, tag="x128")
                nc.sync.dma_start(
                    x128[:], x_d[:].rearrange("b (g f) -> (b g) f", g=8)
                )

                blk_wb = [None] * NBLK
                blk_wb[0] = emit_load(0)
                if NBLK > 1:
                    blk_wb[1] = emit_load(1, tail=(NBLK == 2))

                # ---- x prep ----
                mx128 = smpool.tile([P, 1], F32, tag="mx128")
                nc.vector.tensor_reduce(
                    mx128[:], x128[:], axis=mybir.AxisListType.X,
                    op=mybir.AluOpType.max, apply_absolute_value=True,
                )
                # cross-partition max via PE transpose then reduce
                mrow_ps = ps_ar.tile([1, P], F32, tag="arow")
                nc.tensor.transpose(mrow_ps[:], mx128[:], ident[:])
                mrow = smpool.tile([1, P], F32, tag="mrow_sb")
                nc.vector.tensor_copy(mrow[:], mrow_ps[:])
                mx = smpool.tile([1, 1], F32, tag="mx")
                nc.vector.tensor_reduce(
                    mx[:], mrow[:], axis=mybir.AxisListType.X,
                    op=mybir.AluOpType.max,
                )
                nc.vector.tensor_scalar(
                    mx[:], mx[:], EPS, None, op0=mybir.AluOpType.max
                )
                # inv_s = M/127 (epilogue scale); s = 127/M via recip+Newton
                inv_s = smpool.tile([1, 1], F32, tag="inv_s")
                nc.vector.tensor_scalar(
                    inv_s[:], mx[:], 1.0 / QMAX, None, op0=mybir.AluOpType.mult
                )
                r0 = smpool.tile([1, 1], F32, tag="r0")
                nc.vector.reciprocal(r0[:], mx[:])
                e1 = smpool.tile([1, 1], F32, tag="e1")
                nc.vector.tensor_tensor(e1[:], mx[:], r0[:],
                                        op=mybir.AluOpType.mult)
                nc.vector.tensor_scalar(
                    e1[:], e1[:], -1.0, 2.0,
                    op0=mybir.AluOpType.mult, op1=mybir.AluOpType.add,
                )
                rm = smpool.tile([1, 1], F32, tag="rm")
                nc.vector.tensor_tensor(rm[:], r0[:], e1[:],
                                        op=mybir.AluOpType.mult)
                s11 = smpool.tile([1, 1], F32, tag="s11")
                nc.vector.tensor_scalar(
                    s11[:], rm[:], QMAX, None, op0=mybir.AluOpType.mult
                )
                # broadcast s and inv_s to [128,1] via K=1 matmuls
                sps = ps_sb.tile([P, 2], F32, tag="sps")
                nc.tensor.matmul(sps[:, 0:1], ones_row[:], s11[:],
                                 start=True, stop=True)
                nc.tensor.matmul(sps[:, 1:2], ones_row[:], inv_s[:],
                                 start=True, stop=True)
                s128 = smpool.tile([P, 2], F32, tag="s128")
                nc.vector.tensor_copy(s128[:], sps[:])

                # xi = round(x*s) (f32 magic round), out bf16, [128,1024]
                xr128 = cpool.tile([P, K // 8], F32, tag="xr128")
                nc.vector.tensor_scalar(
                    xr128[:], x128[:], s128[:, 0:1], MAGIC32,
                    op0=mybir.AluOpType.mult, op1=mybir.AluOpType.add,
                )
                xi_nat = xpool.tile([P, K // 8], BF16, tag="xi_nat")
                nc.vector.tensor_scalar(
                    xi_nat[:], xr128[:], MAGIC32, None,
                    op0=mybir.AluOpType.subtract
                )
                # transpose [bg, f1*128+f0] -> xi_t[f0, f1*128 + b*8 + g]
                xi_t = xpool.tile([P, KT * B], BF16, tag="xi_t")
                xps = ps_t.tile([P, 8 * P], BF16, tag="pst", name="xps")
                for f1 in range(8):
                    nc.tensor.transpose(
                        xps[:, f1 * P:(f1 + 1) * P],
                        xi_nat[:, f1 * P:(f1 + 1) * P],
                        identb[:],
                    )
                nc.vector.tensor_copy(xi_t[:], xps[:])
                # moving operand for k-tile kt=(g*8+f1): [128, 16] slice
                xi_v = xi_t[:].rearrange("p (f1 b g) -> p f1 g b",
                                         f1=8, b=B, g=8)

                # negcorr[1,b] = -192 * sum_k xi[b,k]: removes the +192
                # quant offset via one k=1 matmul into the psgT group
                rs = smpool.tile([P, 1], F32, tag="rs")
                nc.vector.tensor_reduce(
                    rs[:], xi_nat[:], axis=mybir.AxisListType.X,
                    op=mybir.AluOpType.add,
                )
                rs_ps = ps_ar.tile([1, P], F32, tag="arow", name="rs_ps")
                nc.tensor.transpose(rs_ps[:], rs[:], ident[:])
                rs_row = smpool.tile([1, P], F32, tag="rs_row")
                nc.vector.tensor_copy(rs_row[:], rs_ps[:])
                negcorr = smpool.tile([1, B], F32, tag="negcorr")
                nc.vector.tensor_reduce(
                    negcorr[:], rs_row[:].rearrange("r (b g) -> r b g", g=8),
                    axis=mybir.AxisListType.X, op=mybir.AluOpType.add,
                )
                nc.vector.tensor_scalar(
                    negcorr[:], negcorr[:], -OFF, None,
                    op0=mybir.AluOpType.mult,
                )
                # correction bank: -192*sum_k xi[b,k] broadcast over o,
                # computed ONCE into its own PSUM bank.  Keeping this f32
                # matmul OUT of the per-block bf16 accumulation groups
                # avoids a mixed-dtype PSUM group (suspected source of a
                # run-to-run precision wobble on HW).
                corr_ps = ps_c.tile([P, B], F32, tag="corr_ps")
                nc.tensor.matmul(corr_ps[:], ones_row[:], negcorr[:],
                                 start=True, stop=True)
                corr_sb = cpool.tile([P, B], F32, tag="corr_sb")
                nc.vector.tensor_copy(corr_sb[:], corr_ps[:])

                # ---- per-block emitters ----
                def act_abs(qb, wb, lo, hi, j, terms):
                    a = apool.tile([P, 1], F32, tag=f"asum{j}",
                                   name=f"asum{j}")
                    if "abs" in skip:
                        nc.vector.memset(a[:], 0.01)
                    else:
                        nc.scalar.activation(
                            qb[:, lo:hi], wb[:, lo:hi],
                            mybir.ActivationFunctionType.Abs,
                            bias=0.0, scale=1.0 / K, accum_out=a[:],
                        )
                    terms.append((a, False))

                def emit_abs_h0(i, wb, tail=False):
                    qb = qpool.tile([P, K], BF16, tag="qb")
                    terms = []
                    act_abs(qb, wb, 0, HALF, 0, terms)
                    return qb, terms

                def emit_abs_h1(i, wb, qb, terms, tail=False):
                    if not tail:
                        act_abs(qb, wb, HALF, K, 1, terms)
                    else:
                        # second half lands as 4 sub-quarter DMAs; chase
                        # them (3 ACT pieces + a DVE tail piece) so alpha
                        # is ready ~2.5us after the last byte
                        act_abs(qb, wb, HALF, 3 * QTR, 1, terms)
                        act_abs(qb, wb, 3 * QTR, 3 * QTR + 1024, 2, terms)
                        cut = 3 * QTR + 1536
                        act_abs(qb, wb, 3 * QTR + 1024, cut, 3, terms)
                        a = apool.tile([P, 1], F32, tag="asum5",
                                       name="asum5")
                        if "abs" in skip:
                            nc.vector.memset(a[:], 0.0)
                        else:
                            nc.vector.tensor_reduce(
                                a[:], wb[:, cut:K],
                                axis=mybir.AxisListType.X,
                                op=mybir.AluOpType.add,
                                apply_absolute_value=True,
                            )
                        terms.append((a, True))

                def emit_alpha(terms):
                    """alpha_sc (= alpha*M/127) and rinv, both [P,1].
                    Only rinv gates the quant; alpha_sc is epilogue-only."""
                    alpha = apool.tile([P, 1], F32, tag="alpha")
                    acc = None
                    for j, (a, raw) in enumerate(terms):
                        t = a
                        if raw:
                            t = apool.tile([P, 1], F32, tag=f"sc{j}",
                                           name=f"sc{j}")
                            nc.vector.tensor_scalar(
                                t[:], a[:], 1.0 / K, None,
                                op0=mybir.AluOpType.mult,
                            )
                        if acc is None:
                            acc = t
                        else:
                            nxt = alpha if j == len(terms) - 1 else \
                                apool.tile([P, 1], F32, tag=f"ac{j}",
                                           name=f"ac{j}")
                            nc.vector.tensor_tensor(
                                nxt[:], acc[:], t[:], op=mybir.AluOpType.add
                            )
                            acc = nxt
                    nc.vector.tensor_scalar(
                        alpha[:], alpha[:], EPS, None, op0=mybir.AluOpType.max
                    )
                    ra = apool.tile([P, 1], F32, tag="ra")
                    nc.vector.reciprocal(ra[:], alpha[:])
                    ea = apool.tile([P, 1], F32, tag="ea")
                    nc.vector.tensor_tensor(
                        ea[:], alpha[:], ra[:], op=mybir.AluOpType.mult
                    )
                    nc.vector.tensor_scalar(
                        ea[:], ea[:], -1.0, 2.0,
                        op0=mybir.AluOpType.mult, op1=mybir.AluOpType.add,
                    )
                    rinv = apool.tile([P, 1], F32, tag="rinv")
                    nc.vector.tensor_tensor(
                        rinv[:], ra[:], ea[:], op=mybir.AluOpType.mult
                    )
                    alpha_sc = apool.tile([P, 1], F32, tag="alpha_sc")
                    nc.vector.tensor_scalar(
                        alpha_sc[:], alpha[:], s128[:, 1:2], None,
                        op0=mybir.AluOpType.mult,
                    )
                    return alpha_sc, rinv

                def process(i, wb, qb, terms):
                    alpha_sc, rinv = emit_alpha(terms)
                    gbuf = gpool.tile([P, KT * P], BF16, tag="gbuf")
                    psg = ps_y.tile([P, B], F32, tag="psg")
                    tailblk = i == NBLK - 1

                    def act_quant(g):
                        # last block only: ACT is free once its abs is
                        # done (no next block), DVE is the tail critical
                        # path -- offload some quant groups
                        return tailblk and g >= 5

                    def emit_mm_group(g):
                        for t in range(8):
                            kt = g * 8 + t
                            g_, f1 = kt // 8, kt % 8
                            nc.tensor.matmul(
                                psg[:],
                                gbuf[:, kt * P:(kt + 1) * P],
                                xi_v[:, f1, g_, :],
                                start=(kt == 0),
                                stop=(kt == KT - 1),
                            )

                    psts = [None] * NG

                    def emit_quant_t(g):
                        lo = g * GW
                        hi = lo + GW
                        if "quant" not in skip:
                            if act_quant(g):
                                nc.scalar.activation(
                                    qb[:, lo:hi], wb[:, lo:hi],
                                    mybir.ActivationFunctionType.Identity,
                                    bias=off192[:], scale=rinv[:],
                                )
                            else:
                                nc.vector.tensor_scalar(
                                    qb[:, lo:hi], wb[:, lo:hi],
                                    rinv[:], OFF,
                                    op0=mybir.AluOpType.mult,
                                    op1=mybir.AluOpType.add,
                                )
                            if _copy_eng(g, tailblk) == "act":
                                # pre-clamp in SBUF (4x-rate bf16). Kept
                                # on DVE: real-HW gpsimd ops proved far
                                # slower than the cost model (PRECLAMP_ENG
                                # knob to re-test)
                                peng = nc.vector if (
                                    tailblk or PRECLAMP_ENG == "dve"
                                ) else nc.gpsimd
                                peng.tensor_scalar(
                                    qb[:, lo:hi], qb[:, lo:hi],
                                    193.0, 191.0,
                                    op0=mybir.AluOpType.min,
                                    op1=mybir.AluOpType.max,
                                )
                        pst = ps_t.tile([P, 8 * P], BF16, tag="pst")
                        psts[g] = pst
                        for t in range(8):
                            nc.tensor.transpose(
                                pst[:, t * P:(t + 1) * P],
                                qb[:, lo + t * P:lo + (t + 1) * P],
                                identb[:],
                            )

                    def emit_copy(g):
                        lo = g * GW
                        hi = lo + GW
                        pst = psts[g]
                        if _copy_eng(g, tailblk) == "act":
                            # f32-bitcast halves the ACT element count.
                            # Value-safe ONLY because pre-clamped: bf16
                            # pairs from {191,192,193} form normal f32s,
                            # and Copy's 1.0*x+0.0 is exact for normals.
                            nc.scalar.copy(
                                gbuf[:, lo:hi].bitcast(F32),
                                pst[:, 0:GW].bitcast(F32),
                            )
                        else:
                            nc.vector.tensor_scalar(
                                gbuf[:, lo:hi], pst[:, 0:GW],
                                193.0, 191.0,
                                op0=mybir.AluOpType.min,
                                op1=mybir.AluOpType.max,
                            )

                    if "mm" in skip:
                        nc.vector.memset(gbuf[:, 0:1], 0.0)
                        nc.tensor.matmul(
                            psg[:], gbuf[:, 0:P], xi_v[:, 0, 0, :],
                            start=True, stop=True,
                        )
                    else:
                        for g in range(NG):
                            emit_quant_t(g)
                            if g >= 1:
                                emit_copy(g - 1)
                            if g >= 2 and not tailblk:
                                emit_mm_group(g - 2)
                        emit_copy(NG - 1)
                        if tailblk:
                            for g in range(NG):
                                emit_mm_group(g)
                        else:
                            emit_mm_group(NG - 2)
                            emit_mm_group(NG - 1)

                    return psg, alpha_sc

                # ---- main loop ----
                # epilogue runs one block late and y stores two blocks
                # late: by then their inputs are long ready, so neither
                # ever head-of-line blocks an in-order engine / the SP
                # weight ring
                def emit_epi(j, psg, alpha_sc):
                    # ysb = alpha_sc * (psg + corr) + bias  (two DVE ops;
                    # runs one block late so nothing ever waits on it)
                    t1 = smpool.tile([P, B], F32, tag="t1")
                    nc.vector.tensor_tensor(
                        t1[:], psg[:], corr_sb[:], op=mybir.AluOpType.add
                    )
                    ysb = smpool.tile([P, B], F32, tag="ysb")
                    nc.vector.tensor_scalar(
                        ysb[:], t1[:], alpha_sc[:], biasc[:, j:j + 1],
                        op0=mybir.AluOpType.mult, op1=mybir.AluOpType.add,
                    )
                    return ysb

                def emit_y_store(j, ysb):
                    nc.sync.dma_start(y_d[j * P:(j + 1) * P, :], ysb[:])

                absd = emit_abs_h0(0, blk_wb[0], tail=(NBLK == 1))
                emit_abs_h1(0, blk_wb[0], *absd, tail=(NBLK == 1))
                pss = [None] * NBLK
                ysbs = [None] * NBLK
                for i in range(NBLK):
                    if i + 2 < NBLK:
                        blk_wb[i + 2] = emit_load(
                            i + 2, tail=(i + 2 == NBLK - 1))
                    if i >= 2:
                        emit_y_store(i - 2, ysbs[i - 2])
                    if i >= 1:
                        ysbs[i - 1] = emit_epi(i - 1, *pss[i - 1])
                    cur = absd
                    if i + 1 < NBLK:
                        absd = emit_abs_h0(i + 1, blk_wb[i + 1],
                                           tail=(i + 1 == NBLK - 1))
                    pss[i] = process(i, blk_wb[i], *cur)
                    if i + 1 < NBLK:
                        emit_abs_h1(i + 1, blk_wb[i + 1], *absd,
                                    tail=(i + 1 == NBLK - 1))
                ysbs[NBLK - 1] = emit_epi(NBLK - 1, *pss[NBLK - 1])
                if NBLK >= 2:
                    emit_y_store(NBLK - 2, ysbs[NBLK - 2])
                emit_y_store(NBLK - 1, ysbs[NBLK - 1])

    nc.finalize()
    return nc


def _get_nc():
    if "nc" not in _CACHE:
        _CACHE["nc"] = _build_bass()
    return _CACHE["nc"]


def _in_maps(x, weight, bias):
    ident = np.eye(P, dtype=np.float32)
    identb = _f32_to_bf16(np.eye(P, dtype=np.float32))
    maps = []
    for c in range(NCORES):
        wsh = np.ascontiguousarray(weight[c * OSH:(c + 1) * OSH])
        bsh = np.ascontiguousarray(
            bias[c * OSH:(c + 1) * OSH].reshape(NBLK, P).T
        )
        maps.append({"x": x, "w": wsh, "biasc": bsh, "ident": ident,
                     "identb": identb})
    return maps


def _f32_to_bf16(a):
    """f32 -> bf16 (RNE) as a uint16-backed ml_dtypes array if available,
    else via jax; run_bass_kernel_spmd wants a real bf16 ndarray."""
    try:
        import ml_dtypes
        return a.astype(ml_dtypes.bfloat16)
    except ImportError:
        import jax.numpy as jnp
        return np.asarray(jnp.asarray(a, dtype=jnp.bfloat16))


def kernel(x, weight, bias):
    x = np.ascontiguousarray(x, dtype=np.float32)
    weight = np.ascontiguousarray(weight, dtype=np.float32)
    bias = np.ascontiguousarray(bias, dtype=np.float32)

    nc = _get_nc()
    os.environ["BASS_NEVER_TRACE"] = "1"
    res = run_bass_kernel_spmd(nc, _in_maps(x, weight, bias),
                               list(range(NCORES)))
    _CACHE["last"] = res
    # y comes back [OSH, B] per core (transposed orientation)
    y = np.concatenate(
        [np.asarray(res.results[c]["y"]).T for c in range(NCORES)], axis=1
    )
    return np.ascontiguousarray(y, dtype=np.float32)


if __name__ == "__main__":
    rng = np.random.default_rng(0)
    x = rng.standard_normal((B, K), dtype=np.float32)
    w = rng.standard_normal((OUT, K), dtype=np.float32) * 0.01
    b = rng.standard_normal(OUT, dtype=np.float32) * 0.01
    y = kernel(x=x, weight=w, bias=b)
    print(y.shape, y.dtype)


# revision 40
# speedup vs baseline: 1.0865x; 1.0865x over previous
"""BitLinear (BitNet b1.58) TP kernel for 8 NeuronCores.

y = fake_quant_act(x) @ ternary_absmean(W).T + bias

Sharding: W (and bias) split along out_features across 8 cores; x replicated.
Per core: W shard [1024, 8192] f32, x [16, 8192] f32 -> y shard [16, 1024].

v5 design (HW-measured ~119-121 us vs ~135 us baseline; DMA floor
~102 us; cost-model/TimelineSim estimate 113.8 us):
  - SP DGE ring carries x first, then ONLY the wb weight stream, plus
    tiny y stores one/two blocks late (their data is long ready) so the
    stream never head-of-line stalls.  Constants ride the ACT ring.
  - Matmul orientation produces y TRANSPOSED: psgT[o,b] += gbuf[k,o].T
    @ xiT[k,b].  alpha / bias are then per-PARTITION, so the entire
    epilogue is one extra k=1 matmul (-192*sum_k xi[b,k] outer ones_o,
    removing the +192 quant offset) followed by a single ACT op
    ysbT = alpha_sc * psgT + bias_col.  (The old orientation needed a
    5-op PSUM-reading epilogue chain per block.)  y is stored [o,b]
    and transposed on the host.
  - Per block (128 rows x 8192 cols), work flows in 8 column-groups:
    DVE quant group (bf16(rinv*w+192), RNE == round) -> PE 8 transposes
    -> PSUM->SBUF copy -> PE 8 matmuls (2-group lag).  Copies: groups
    0,2 via ACT (f32-bitcast copy of a DVE-pre-clamped source), rest
    via DVE with the clamp to {191,192,193} fused.  NO gpsimd anywhere:
    Pool cannot touch PSUM, and its SBUF ops measured far slower on HW
    than the cost model says (+12 us total when used for pre-clamps).
  - abs row-sums (for alpha) on ACT in halves, emitted adjacent so ACT
    chases the DMA lands; the LAST block's second half lands as 4
    quarter-DMAs chased by 3 ACT pieces + a DVE tail piece, and its
    quant is split DVE(g0-g4)/ACT(g5-g7, activation Identity with
    per-row scale) to shorten the serial tail.
  - Measured-slower on HW (despite equal cost-model time): gpsimd
    pre-clamps/copies/epilogue; 4-way tail copy split + split tail h0
    (138 us); NG=16 groups; x on the ACT ring; moving the offset
    correction out of the matmul group at ps_t=3 (134 us).
"""

import contextlib
import os
import numpy as np

import concourse.bass as bass
import concourse.bacc as bacc
import concourse.mybir as mybir
from concourse import tile
from concourse.bass_utils import run_bass_kernel_spmd

F32 = mybir.dt.float32
BF16 = mybir.dt.bfloat16

B = 16          # batch
K = 8192        # in_features
OUT = 8192      # out_features
NCORES = 8
OSH = OUT // NCORES      # 1024 out rows per core
P = 128
NBLK = OSH // P          # 8 o-blocks per core
KT = K // P              # 64 k-tiles
NG = 8                   # col groups per block (8 k-tiles = 1024 cols each)
GW = K // NG             # 1024 cols per group
EPS = 1e-8
QMAX = 127.0
MAGIC32 = 1.5 * 2.0**23  # f32 round-to-nearest-even magic
OFF = 192.0              # bf16 round magic (ulp=1 in [128,256))

HALF = K // 2
QTR = K // 4

_CACHE = {}
PRECLAMP_ENG = "dve"   # "pool" to offload mid-stream pre-clamps to gpsimd


def _copy_eng(g, tail=False):
    # PSUM->SBUF copy engine per group: ACT (bitcast f32, pre-clamped
    # source) for even-ish groups, DVE (clamp fused) otherwise
    if tail:
        return "act" if g in (0, 2) else "dve"
    return "act" if g in (0, 2) else "dve"


def _build_bass(loop_r=None, py_r=1, skip=()):
    nc = bacc.Bacc()
    x_d = nc.declare_dram_parameter("x", [B, K], F32, isOutput=False)
    w_d = nc.declare_dram_parameter("w", [OSH, K], F32, isOutput=False)
    bc_d = nc.declare_dram_parameter("biasc", [P, NBLK], F32, isOutput=False)
    id_d = nc.declare_dram_parameter("ident", [P, P], F32, isOutput=False)
    idb_d = nc.declare_dram_parameter("identb", [P, P], BF16, isOutput=False)
    y_d = nc.declare_dram_parameter("y", [OSH, B], F32, isOutput=True)

    with tile.TileContext(nc) as tc:
        with (
            tc.tile_pool(name="const", bufs=1) as cpool,
            tc.tile_pool(name="wp", bufs=3) as wpool,
            tc.tile_pool(name="qp", bufs=3) as qpool,
            tc.tile_pool(name="gp", bufs=2) as gpool,
            tc.tile_pool(name="sm", bufs=2) as smpool,
            tc.tile_pool(name="xp", bufs=2) as xpool,
            tc.tile_pool(name="al", bufs=4) as apool,
            tc.tile_pool(name="ps_y", bufs=2, space="PSUM") as ps_y,
            tc.tile_pool(name="ps_t", bufs=4, space="PSUM") as ps_t,
            tc.tile_pool(name="ps_ar", bufs=1, space="PSUM") as ps_ar,
            tc.tile_pool(name="ps_sb", bufs=1, space="PSUM") as ps_sb,
        ):
            # ---- constants (one-time, ACT ring) ----
            ident = cpool.tile([P, P], F32, tag="ident")
            nc.scalar.dma_start(ident[:], id_d[:])
            identb = cpool.tile([P, P], BF16, tag="identb")
            nc.scalar.dma_start(identb[:], idb_d[:])
            ones_row = cpool.tile([1, P], F32, tag="ones_row")
            nc.vector.memset(ones_row[:], 1.0)
            off192 = cpool.tile([P, 1], F32, tag="off192")
            nc.vector.memset(off192[:], OFF)
            biasc = cpool.tile([P, NBLK], F32, tag="biasc")
            nc.scalar.dma_start(biasc[:], bc_d[:])

            loop = tc.For_i(0, loop_r) if loop_r else contextlib.nullcontext()
            for _py_rep in range(py_r):
              with loop:
                # ---- weight stream (SP ring) ----
                def emit_load(i, tail=False):
                    wb = wpool.tile([P, K], F32, tag="wb")
                    cuts = [0, HALF, K] if not tail else \
                        [0, HALF, HALF + QTR // 2, 3 * QTR,
                         3 * QTR + QTR // 2, K]
                    for lo, hi in zip(cuts[:-1], cuts[1:]):
                        nc.sync.dma_start(
                            wb[:, lo:hi],
                            w_d[i * P:(i + 1) * P, lo:hi],
                        )
                    return wb

                # x rides the SP ring FIRST (tiny, no deps, and the whole
                # x-prep chain sits ahead of block 0's alpha in DVE
                # program order -- a late x would stall every block).
                # load x as [128, 1024]: partition = (b*8+g), free = f,
                # k = g*1024+f
                x128 = cpool.tile([P, K // 8], F32, tag="x128")
                nc.sync.dma_start(
                    x128[:], x_d[:].rearrange("b (g f) -> (b g) f", g=8)
                )

                blk_wb = [None] * NBLK
                blk_wb[0] = emit_load(0)
                if NBLK > 1:
                    blk_wb[1] = emit_load(1, tail=(NBLK == 2))

                # ---- x prep ----
                mx128 = smpool.tile([P, 1], F32, tag="mx128")
                nc.vector.tensor_reduce(
                    mx128[:], x128[:], axis=mybir.AxisListType.X,
                    op=mybir.AluOpType.max, apply_absolute_value=True,
                )
                # cross-partition max via PE transpose then reduce
                mrow_ps = ps_ar.tile([1, P], F32, tag="arow")
                nc.tensor.transpose(mrow_ps[:], mx128[:], ident[:])
                mrow = smpool.tile([1, P], F32, tag="mrow_sb")
                nc.vector.tensor_copy(mrow[:], mrow_ps[:])
                mx = smpool.tile([1, 1], F32, tag="mx")
                nc.vector.tensor_reduce(
                    mx[:], mrow[:], axis=mybir.AxisListType.X,
                    op=mybir.AluOpType.max,
                )
                nc.vector.tensor_scalar(
                    mx[:], mx[:], EPS, None, op0=mybir.AluOpType.max
                )
                # inv_s = M/127 (epilogue scale); s = 127/M via recip+Newton
                inv_s = smpool.tile([1, 1], F32, tag="inv_s")
                nc.vector.tensor_scalar(
                    inv_s[:], mx[:], 1.0 / QMAX, None, op0=mybir.AluOpType.mult
                )
                r0 = smpool.tile([1, 1], F32, tag="r0")
                nc.vector.reciprocal(r0[:], mx[:])
                e1 = smpool.tile([1, 1], F32, tag="e1")
                nc.vector.tensor_tensor(e1[:], mx[:], r0[:],
                                        op=mybir.AluOpType.mult)
                nc.vector.tensor_scalar(
                    e1[:], e1[:], -1.0, 2.0,
                    op0=mybir.AluOpType.mult, op1=mybir.AluOpType.add,
                )
                rm = smpool.tile([1, 1], F32, tag="rm")
                nc.vector.tensor_tensor(rm[:], r0[:], e1[:],
                                        op=mybir.AluOpType.mult)
                s11 = smpool.tile([1, 1], F32, tag="s11")
                nc.vector.tensor_scalar(
                    s11[:], rm[:], QMAX, None, op0=mybir.AluOpType.mult
                )
                # broadcast s and inv_s to [128,1] via K=1 matmuls
                sps = ps_sb.tile([P, 2], F32, tag="sps")
                nc.tensor.matmul(sps[:, 0:1], ones_row[:], s11[:],
                                 start=True, stop=True)
                nc.tensor.matmul(sps[:, 1:2], ones_row[:], inv_s[:],
                                 start=True, stop=True)
                s128 = smpool.tile([P, 2], F32, tag="s128")
                nc.vector.tensor_copy(s128[:], sps[:])

                # xi = round(x*s) (f32 magic round), out bf16, [128,1024]
                xr128 = cpool.tile([P, K // 8], F32, tag="xr128")
                nc.vector.tensor_scalar(
                    xr128[:], x128[:], s128[:, 0:1], MAGIC32,
                    op0=mybir.AluOpType.mult, op1=mybir.AluOpType.add,
                )
                xi_nat = xpool.tile([P, K // 8], BF16, tag="xi_nat")
                nc.vector.tensor_scalar(
                    xi_nat[:], xr128[:], MAGIC32, None,
                    op0=mybir.AluOpType.subtract
                )
                # transpose [bg, f1*128+f0] -> xi_t[f0, f1*128 + b*8 + g]
                xi_t = xpool.tile([P, KT * B], BF16, tag="xi_t")
                xps = ps_t.tile([P, 8 * P], BF16, tag="pst", name="xps")
                for f1 in range(8):
                    nc.tensor.transpose(
                        xps[:, f1 * P:(f1 + 1) * P],
                        xi_nat[:, f1 * P:(f1 + 1) * P],
                        identb[:],
                    )
                nc.vector.tensor_copy(xi_t[:], xps[:])
                # moving operand for k-tile kt=(g*8+f1): [128, 16] slice
                xi_v = xi_t[:].rearrange("p (f1 b g) -> p f1 g b",
                                         f1=8, b=B, g=8)

                # negcorr[1,b] = -192 * sum_k xi[b,k]: removes the +192
                # quant offset via one k=1 matmul into the psgT group
                rs = smpool.tile([P, 1], F32, tag="rs")
                nc.vector.tensor_reduce(
                    rs[:], xi_nat[:], axis=mybir.AxisListType.X,
                    op=mybir.AluOpType.add,
                )
                rs_ps = ps_ar.tile([1, P], F32, tag="arow", name="rs_ps")
                nc.tensor.transpose(rs_ps[:], rs[:], ident[:])
                rs_row = smpool.tile([1, P], F32, tag="rs_row")
                nc.vector.tensor_copy(rs_row[:], rs_ps[:])
                negcorr = smpool.tile([1, B], F32, tag="negcorr")
                nc.vector.tensor_reduce(
                    negcorr[:], rs_row[:].rearrange("r (b g) -> r b g", g=8),
                    axis=mybir.AxisListType.X, op=mybir.AluOpType.add,
                )
                nc.vector.tensor_scalar(
                    negcorr[:], negcorr[:], -OFF, None,
                    op0=mybir.AluOpType.mult,
                )

                # ---- per-block emitters ----
                def act_abs(qb, wb, lo, hi, j, terms):
                    a = apool.tile([P, 1], F32, tag=f"asum{j}",
                                   name=f"asum{j}")
                    if "abs" in skip:
                        nc.vector.memset(a[:], 0.01)
                    else:
                        nc.scalar.activation(
                            qb[:, lo:hi], wb[:, lo:hi],
                            mybir.ActivationFunctionType.Abs,
                            bias=0.0, scale=1.0 / K, accum_out=a[:],
                        )
                    terms.append((a, False))

                def emit_abs_h0(i, wb, tail=False):
                    qb = qpool.tile([P, K], BF16, tag="qb")
                    terms = []
                    act_abs(qb, wb, 0, HALF, 0, terms)
                    return qb, terms

                def emit_abs_h1(i, wb, qb, terms, tail=False):
                    if not tail:
                        act_abs(qb, wb, HALF, K, 1, terms)
                    else:
                        # second half lands as 4 sub-quarter DMAs; chase
                        # them (3 ACT pieces + a DVE tail piece) so alpha
                        # is ready ~2.5us after the last byte
                        act_abs(qb, wb, HALF, 3 * QTR, 1, terms)
                        act_abs(qb, wb, 3 * QTR, 3 * QTR + 1024, 2, terms)
                        cut = 3 * QTR + 1536
                        act_abs(qb, wb, 3 * QTR + 1024, cut, 3, terms)
                        a = apool.tile([P, 1], F32, tag="asum5",
                                       name="asum5")
                        if "abs" in skip:
                            nc.vector.memset(a[:], 0.0)
                        else:
                            nc.vector.tensor_reduce(
                                a[:], wb[:, cut:K],
                                axis=mybir.AxisListType.X,
                                op=mybir.AluOpType.add,
                                apply_absolute_value=True,
                            )
                        terms.append((a, True))

                def emit_alpha(terms):
                    """alpha_sc (= alpha*M/127) and rinv, both [P,1].
                    Only rinv gates the quant; alpha_sc is epilogue-only."""
                    alpha = apool.tile([P, 1], F32, tag="alpha")
                    acc = None
                    for j, (a, raw) in enumerate(terms):
                        t = a
                        if raw:
                            t = apool.tile([P, 1], F32, tag=f"sc{j}",
                                           name=f"sc{j}")
                            nc.vector.tensor_scalar(
                                t[:], a[:], 1.0 / K, None,
                                op0=mybir.AluOpType.mult,
                            )
                        if acc is None:
                            acc = t
                        else:
                            nxt = alpha if j == len(terms) - 1 else \
                                apool.tile([P, 1], F32, tag=f"ac{j}",
                                           name=f"ac{j}")
                            nc.vector.tensor_tensor(
                                nxt[:], acc[:], t[:], op=mybir.AluOpType.add
                            )
                            acc = nxt
                    nc.vector.tensor_scalar(
                        alpha[:], alpha[:], EPS, None, op0=mybir.AluOpType.max
                    )
                    ra = apool.tile([P, 1], F32, tag="ra")
                    nc.vector.reciprocal(ra[:], alpha[:])
                    ea = apool.tile([P, 1], F32, tag="ea")
                    nc.vector.tensor_tensor(
                        ea[:], alpha[:], ra[:], op=mybir.AluOpType.mult
                    )
                    nc.vector.tensor_scalar(
                        ea[:], ea[:], -1.0, 2.0,
                        op0=mybir.AluOpType.mult, op1=mybir.AluOpType.add,
                    )
                    rinv = apool.tile([P, 1], F32, tag="rinv")
                    nc.vector.tensor_tensor(
                        rinv[:], ra[:], ea[:], op=mybir.AluOpType.mult
                    )
                    alpha_sc = apool.tile([P, 1], F32, tag="alpha_sc")
                    nc.vector.tensor_scalar(
                        alpha_sc[:], alpha[:], s128[:, 1:2], None,
                        op0=mybir.AluOpType.mult,
                    )
                    return alpha_sc, rinv

                def process(i, wb, qb, terms):
                    alpha_sc, rinv = emit_alpha(terms)
                    gbuf = gpool.tile([P, KT * P], BF16, tag="gbuf")
                    psg = ps_y.tile([P, B], F32, tag="psg")
                    tailblk = i == NBLK - 1

                    def act_quant(g):
                        # last block only: ACT is free once its abs is
                        # done (no next block), DVE is the tail critical
                        # path -- offload some quant groups
                        return tailblk and g >= 5

                    def emit_mm_group(g):
                        for t in range(8):
                            kt = g * 8 + t
                            g_, f1 = kt // 8, kt % 8
                            nc.tensor.matmul(
                                psg[:],
                                gbuf[:, kt * P:(kt + 1) * P],
                                xi_v[:, f1, g_, :],
                                start=(kt == 0),
                                stop=False,
                            )

                    def emit_mm_corr():
                        # psgT -= 192*sum_k xi[b,k] (outer ones_o): undoes
                        # the +192 quant offset, f32 k=1 matmul
                        nc.tensor.matmul(
                            psg[:], ones_row[:], negcorr[:],
                            start=False, stop=True,
                        )

                    psts = [None] * NG

                    def emit_quant_t(g):
                        lo = g * GW
                        hi = lo + GW
                        if "quant" not in skip:
                            if act_quant(g):
                                nc.scalar.activation(
                                    qb[:, lo:hi], wb[:, lo:hi],
                                    mybir.ActivationFunctionType.Identity,
                                    bias=off192[:], scale=rinv[:],
                                )
                            else:
                                nc.vector.tensor_scalar(
                                    qb[:, lo:hi], wb[:, lo:hi],
                                    rinv[:], OFF,
                                    op0=mybir.AluOpType.mult,
                                    op1=mybir.AluOpType.add,
                                )
                            if _copy_eng(g, tailblk) == "act":
                                # pre-clamp in SBUF (4x-rate bf16). Kept
                                # on DVE: real-HW gpsimd ops proved far
                                # slower than the cost model (PRECLAMP_ENG
                                # knob to re-test)
                                peng = nc.vector if (
                                    tailblk or PRECLAMP_ENG == "dve"
                                ) else nc.gpsimd
                                peng.tensor_scalar(
                                    qb[:, lo:hi], qb[:, lo:hi],
                                    193.0, 191.0,
                                    op0=mybir.AluOpType.min,
                                    op1=mybir.AluOpType.max,
                                )
                        pst = ps_t.tile([P, 8 * P], BF16, tag="pst")
                        psts[g] = pst
                        for t in range(8):
                            nc.tensor.transpose(
                                pst[:, t * P:(t + 1) * P],
                                qb[:, lo + t * P:lo + (t + 1) * P],
                                identb[:],
                            )

                    def emit_copy(g):
                        lo = g * GW
                        hi = lo + GW
                        pst = psts[g]
                        if _copy_eng(g, tailblk) == "act":
                            # f32-bitcast halves the ACT element count.
                            # Value-safe ONLY because pre-clamped: bf16
                            # pairs from {191,192,193} form normal f32s,
                            # and Copy's 1.0*x+0.0 is exact for normals.
                            nc.scalar.copy(
                                gbuf[:, lo:hi].bitcast(F32),
                                pst[:, 0:GW].bitcast(F32),
                            )
                        else:
                            nc.vector.tensor_scalar(
                                gbuf[:, lo:hi], pst[:, 0:GW],
                                193.0, 191.0,
                                op0=mybir.AluOpType.min,
                                op1=mybir.AluOpType.max,
                            )

                    if "mm" in skip:
                        nc.vector.memset(gbuf[:, 0:1], 0.0)
                        nc.tensor.matmul(
                            psg[:], gbuf[:, 0:P], xi_v[:, 0, 0, :],
                            start=True, stop=True,
                        )
                    else:
                        for g in range(NG):
                            emit_quant_t(g)
                            if g >= 1:
                                emit_copy(g - 1)
                            if g >= 2 and not tailblk:
                                emit_mm_group(g - 2)
                        emit_copy(NG - 1)
                        if tailblk:
                            for g in range(NG):
                                emit_mm_group(g)
                        else:
                            emit_mm_group(NG - 2)
                            emit_mm_group(NG - 1)
                        emit_mm_corr()

                    return psg, alpha_sc

                # ---- main loop ----
                # epilogue runs one block late and y stores two blocks
                # late: by then their inputs are long ready, so neither
                # ever head-of-line blocks an in-order engine / the SP
                # weight ring
                def emit_epi(j, psg, alpha_sc):
                    ysb = smpool.tile([P, B], F32, tag="ysb")
                    nc.scalar.activation(
                        ysb[:], psg[:],
                        mybir.ActivationFunctionType.Identity,
                        bias=biasc[:, j:j + 1], scale=alpha_sc[:],
                    )
                    return ysb

                def emit_y_store(j, ysb):
                    nc.sync.dma_start(y_d[j * P:(j + 1) * P, :], ysb[:])

                absd = emit_abs_h0(0, blk_wb[0], tail=(NBLK == 1))
                emit_abs_h1(0, blk_wb[0], *absd, tail=(NBLK == 1))
                pss = [None] * NBLK
                ysbs = [None] * NBLK
                for i in range(NBLK):
                    if i + 2 < NBLK:
                        blk_wb[i + 2] = emit_load(
                            i + 2, tail=(i + 2 == NBLK - 1))
                    if i >= 2:
                        emit_y_store(i - 2, ysbs[i - 2])
                    if i >= 1:
                        ysbs[i - 1] = emit_epi(i - 1, *pss[i - 1])
                    cur = absd
                    if i + 1 < NBLK:
                        absd = emit_abs_h0(i + 1, blk_wb[i + 1],
                                           tail=(i + 1 == NBLK - 1))
                    pss[i] = process(i, blk_wb[i], *cur)
                    if i + 1 < NBLK:
                        emit_abs_h1(i + 1, blk_wb[i + 1], *absd,
                                    tail=(i + 1 == NBLK - 1))
                ysbs[NBLK - 1] = emit_epi(NBLK - 1, *pss[NBLK - 1])
                if NBLK >= 2:
                    emit_y_store(NBLK - 2, ysbs[NBLK - 2])
                emit_y_store(NBLK - 1, ysbs[NBLK - 1])

    nc.finalize()
    return nc


def _get_nc():
    if "nc" not in _CACHE:
        _CACHE["nc"] = _build_bass()
    return _CACHE["nc"]


def _in_maps(x, weight, bias):
    ident = np.eye(P, dtype=np.float32)
    identb = _f32_to_bf16(np.eye(P, dtype=np.float32))
    maps = []
    for c in range(NCORES):
        wsh = np.ascontiguousarray(weight[c * OSH:(c + 1) * OSH])
        bsh = np.ascontiguousarray(
            bias[c * OSH:(c + 1) * OSH].reshape(NBLK, P).T
        )
        maps.append({"x": x, "w": wsh, "biasc": bsh, "ident": ident,
                     "identb": identb})
    return maps


def _f32_to_bf16(a):
    """f32 -> bf16 (RNE) as a uint16-backed ml_dtypes array if available,
    else via jax; run_bass_kernel_spmd wants a real bf16 ndarray."""
    try:
        import ml_dtypes
        return a.astype(ml_dtypes.bfloat16)
    except ImportError:
        import jax.numpy as jnp
        return np.asarray(jnp.asarray(a, dtype=jnp.bfloat16))


def kernel(x, weight, bias):
    x = np.ascontiguousarray(x, dtype=np.float32)
    weight = np.ascontiguousarray(weight, dtype=np.float32)
    bias = np.ascontiguousarray(bias, dtype=np.float32)

    nc = _get_nc()
    os.environ["BASS_NEVER_TRACE"] = "1"
    res = run_bass_kernel_spmd(nc, _in_maps(x, weight, bias),
                               list(range(NCORES)))
    _CACHE["last"] = res
    # y comes back [OSH, B] per core (transposed orientation)
    y = np.concatenate(
        [np.asarray(res.results[c]["y"]).T for c in range(NCORES)], axis=1
    )
    return np.ascontiguousarray(y, dtype=np.float32)


if __name__ == "__main__":
    rng = np.random.default_rng(0)
    x = rng.standard_normal((B, K), dtype=np.float32)
    w = rng.standard_normal((OUT, K), dtype=np.float32) * 0.01
    b = rng.standard_normal(OUT, dtype=np.float32) * 0.01
    y = kernel(x=x, weight=w, bias=b)
    print(y.shape, y.dtype)


# revision 42
# speedup vs baseline: 1.1178x; 1.0289x over previous
"""BitLinear (BitNet b1.58) TP kernel for 8 NeuronCores.

y = fake_quant_act(x) @ ternary_absmean(W).T + bias

Sharding: W (and bias) split along out_features across 8 cores; x replicated.
Per core: W shard [1024, 8192] f32, x [16, 8192] f32 -> y shard [16, 1024].

v5 design (HW-measured ~119-121 us vs ~135 us baseline; DMA floor
~102 us; cost-model/TimelineSim estimate 113.8 us):
  - SP DGE ring carries x first, then ONLY the wb weight stream, plus
    tiny y stores one/two blocks late (their data is long ready) so the
    stream never head-of-line stalls.  Constants ride the ACT ring.
  - Matmul orientation produces y TRANSPOSED: psgT[o,b] += gbuf[k,o].T
    @ xiT[k,b].  alpha / bias are then per-PARTITION, so the entire
    epilogue is one extra k=1 matmul (-192*sum_k xi[b,k] outer ones_o,
    removing the +192 quant offset) followed by a single ACT op
    ysbT = alpha_sc * psgT + bias_col.  (The old orientation needed a
    5-op PSUM-reading epilogue chain per block.)  y is stored [o,b]
    and transposed on the host.
  - Per block (128 rows x 8192 cols), work flows in 8 column-groups:
    DVE quant group (bf16(rinv*w+192), RNE == round) -> PE 8 transposes
    -> PSUM->SBUF copy -> PE 8 matmuls (2-group lag).  Copies: groups
    0,2 via ACT (f32-bitcast copy of a DVE-pre-clamped source), rest
    via DVE with the clamp to {191,192,193} fused.  NO gpsimd anywhere:
    Pool cannot touch PSUM, and its SBUF ops measured far slower on HW
    than the cost model says (+12 us total when used for pre-clamps).
  - abs row-sums (for alpha) on ACT in halves, emitted adjacent so ACT
    chases the DMA lands; the LAST block's second half lands as 4
    quarter-DMAs chased by 3 ACT pieces + a DVE tail piece, and its
    quant is split DVE(g0-g4)/ACT(g5-g7, activation Identity with
    per-row scale) to shorten the serial tail.
  - Measured-slower on HW (despite equal cost-model time): gpsimd
    pre-clamps/copies/epilogue; 4-way tail copy split + split tail h0
    (138 us); NG=16 groups; x on the ACT ring; moving the offset
    correction out of the matmul group at ps_t=3 (134 us).
"""

import contextlib
import os
import numpy as np

import concourse.bass as bass
import concourse.bacc as bacc
import concourse.mybir as mybir
from concourse import tile
from concourse.bass_utils import run_bass_kernel_spmd

F32 = mybir.dt.float32
BF16 = mybir.dt.bfloat16

B = 16          # batch
K = 8192        # in_features
OUT = 8192      # out_features
NCORES = 8
OSH = OUT // NCORES      # 1024 out rows per core
P = 128
NBLK = OSH // P          # 8 o-blocks per core
KT = K // P              # 64 k-tiles
NG = 8                   # col groups per block (8 k-tiles = 1024 cols each)
GW = K // NG             # 1024 cols per group
EPS = 1e-8
QMAX = 127.0
MAGIC32 = 1.5 * 2.0**23  # f32 round-to-nearest-even magic
OFF = 192.0              # bf16 round magic (ulp=1 in [128,256))

HALF = K // 2
QTR = K // 4

_CACHE = {}
PRECLAMP_ENG = "dve"   # "pool" to offload mid-stream pre-clamps to gpsimd


def _copy_eng(g, tail=False):
    # PSUM->SBUF copy engine per group: ACT (bitcast f32, pre-clamped
    # source) for even-ish groups, DVE (clamp fused) otherwise
    if tail:
        return "act" if g in (0, 2) else "dve"
    return "act" if g in (0, 2) else "dve"


def _build_bass(loop_r=None, py_r=1, skip=()):
    nc = bacc.Bacc()
    x_d = nc.declare_dram_parameter("x", [B, K], F32, isOutput=False)
    w_d = nc.declare_dram_parameter("w", [OSH, K], F32, isOutput=False)
    bc_d = nc.declare_dram_parameter("biasc", [P, NBLK], F32, isOutput=False)
    id_d = nc.declare_dram_parameter("ident", [P, P], F32, isOutput=False)
    idb_d = nc.declare_dram_parameter("identb", [P, P], BF16, isOutput=False)
    y_d = nc.declare_dram_parameter("y", [OSH, B], F32, isOutput=True)

    with tile.TileContext(nc) as tc:
        with (
            tc.tile_pool(name="const", bufs=1) as cpool,
            tc.tile_pool(name="wp", bufs=3) as wpool,
            tc.tile_pool(name="qp", bufs=3) as qpool,
            tc.tile_pool(name="gp", bufs=2) as gpool,
            tc.tile_pool(name="sm", bufs=2) as smpool,
            tc.tile_pool(name="xp", bufs=2) as xpool,
            tc.tile_pool(name="al", bufs=4) as apool,
            tc.tile_pool(name="ps_y", bufs=2, space="PSUM") as ps_y,
            tc.tile_pool(name="ps_t", bufs=4, space="PSUM") as ps_t,
            tc.tile_pool(name="ps_ar", bufs=1, space="PSUM") as ps_ar,
            tc.tile_pool(name="ps_sb", bufs=1, space="PSUM") as ps_sb,
        ):
            # ---- constants (one-time, ACT ring) ----
            ident = cpool.tile([P, P], F32, tag="ident")
            nc.scalar.dma_start(ident[:], id_d[:])
            identb = cpool.tile([P, P], BF16, tag="identb")
            nc.scalar.dma_start(identb[:], idb_d[:])
            ones_row = cpool.tile([1, P], F32, tag="ones_row")
            nc.vector.memset(ones_row[:], 1.0)
            off192 = cpool.tile([P, 1], F32, tag="off192")
            nc.vector.memset(off192[:], OFF)
            biasc = cpool.tile([P, NBLK], F32, tag="biasc")
            nc.scalar.dma_start(biasc[:], bc_d[:])

            loop = tc.For_i(0, loop_r) if loop_r else contextlib.nullcontext()
            for _py_rep in range(py_r):
              with loop:
                # ---- weight stream (SP ring) ----
                def emit_load(i, tail=False):
                    wb = wpool.tile([P, K], F32, tag="wb")
                    cuts = [0, HALF, K] if not tail else \
                        [0, HALF, HALF + QTR // 2, 3 * QTR,
                         3 * QTR + QTR // 2, K]
                    for lo, hi in zip(cuts[:-1], cuts[1:]):
                        nc.sync.dma_start(
                            wb[:, lo:hi],
                            w_d[i * P:(i + 1) * P, lo:hi],
                        )
                    return wb

                # x rides the SP ring FIRST (tiny, no deps, and the whole
                # x-prep chain sits ahead of block 0's alpha in DVE
                # program order -- a late x would stall every block).
                # load x as [128, 1024]: partition = (b*8+g), free = f,
                # k = g*1024+f
                x128 = cpool.tile([P, K // 8], F32, tag="x128")
                nc.sync.dma_start(
                    x128[:], x_d[:].rearrange("b (g f) -> (b g) f", g=8)
                )

                blk_wb = [None] * NBLK
                blk_wb[0] = emit_load(0)
                if NBLK > 1:
                    blk_wb[1] = emit_load(1, tail=(NBLK == 2))

                # ---- x prep ----
                mx128 = smpool.tile([P, 1], F32, tag="mx128")
                nc.vector.tensor_reduce(
                    mx128[:], x128[:], axis=mybir.AxisListType.X,
                    op=mybir.AluOpType.max, apply_absolute_value=True,
                )
                # cross-partition max via PE transpose then reduce
                mrow_ps = ps_ar.tile([1, P], F32, tag="arow")
                nc.tensor.transpose(mrow_ps[:], mx128[:], ident[:])
                mrow = smpool.tile([1, P], F32, tag="mrow_sb")
                nc.vector.tensor_copy(mrow[:], mrow_ps[:])
                mx = smpool.tile([1, 1], F32, tag="mx")
                nc.vector.tensor_reduce(
                    mx[:], mrow[:], axis=mybir.AxisListType.X,
                    op=mybir.AluOpType.max,
                )
                nc.vector.tensor_scalar(
                    mx[:], mx[:], EPS, None, op0=mybir.AluOpType.max
                )
                # inv_s = M/127 (epilogue scale); s = 127/M via recip+Newton
                inv_s = smpool.tile([1, 1], F32, tag="inv_s")
                nc.vector.tensor_scalar(
                    inv_s[:], mx[:], 1.0 / QMAX, None, op0=mybir.AluOpType.mult
                )
                r0 = smpool.tile([1, 1], F32, tag="r0")
                nc.vector.reciprocal(r0[:], mx[:])
                e1 = smpool.tile([1, 1], F32, tag="e1")
                nc.vector.tensor_tensor(e1[:], mx[:], r0[:],
                                        op=mybir.AluOpType.mult)
                nc.vector.tensor_scalar(
                    e1[:], e1[:], -1.0, 2.0,
                    op0=mybir.AluOpType.mult, op1=mybir.AluOpType.add,
                )
                rm = smpool.tile([1, 1], F32, tag="rm")
                nc.vector.tensor_tensor(rm[:], r0[:], e1[:],
                                        op=mybir.AluOpType.mult)
                s11 = smpool.tile([1, 1], F32, tag="s11")
                nc.vector.tensor_scalar(
                    s11[:], rm[:], QMAX, None, op0=mybir.AluOpType.mult
                )
                # broadcast s and inv_s to [128,1] via K=1 matmuls
                sps = ps_sb.tile([P, 2], F32, tag="sps")
                nc.tensor.matmul(sps[:, 0:1], ones_row[:], s11[:],
                                 start=True, stop=True)
                nc.tensor.matmul(sps[:, 1:2], ones_row[:], inv_s[:],
                                 start=True, stop=True)
                s128 = smpool.tile([P, 2], F32, tag="s128")
                nc.vector.tensor_copy(s128[:], sps[:])

                # xi = round(x*s) (f32 magic round), out bf16, [128,1024]
                xr128 = cpool.tile([P, K // 8], F32, tag="xr128")
                nc.vector.tensor_scalar(
                    xr128[:], x128[:], s128[:, 0:1], MAGIC32,
                    op0=mybir.AluOpType.mult, op1=mybir.AluOpType.add,
                )
                xi_nat = xpool.tile([P, K // 8], BF16, tag="xi_nat")
                nc.vector.tensor_scalar(
                    xi_nat[:], xr128[:], MAGIC32, None,
                    op0=mybir.AluOpType.subtract
                )
                # transpose [bg, f1*128+f0] -> xi_t[f0, f1*128 + b*8 + g]
                xi_t = xpool.tile([P, KT * B], BF16, tag="xi_t")
                xps = ps_t.tile([P, 8 * P], BF16, tag="pst", name="xps")
                for f1 in range(8):
                    nc.tensor.transpose(
                        xps[:, f1 * P:(f1 + 1) * P],
                        xi_nat[:, f1 * P:(f1 + 1) * P],
                        identb[:],
                    )
                nc.vector.tensor_copy(xi_t[:], xps[:])
                # moving operand for k-tile kt=(g*8+f1): [128, 16] slice
                xi_v = xi_t[:].rearrange("p (f1 b g) -> p f1 g b",
                                         f1=8, b=B, g=8)

                # negcorr[1,b] = -192 * sum_k xi[b,k]: removes the +192
                # quant offset via one k=1 matmul into the psgT group
                rs = smpool.tile([P, 1], F32, tag="rs")
                nc.vector.tensor_reduce(
                    rs[:], xi_nat[:], axis=mybir.AxisListType.X,
                    op=mybir.AluOpType.add,
                )
                rs_ps = ps_ar.tile([1, P], F32, tag="arow", name="rs_ps")
                nc.tensor.transpose(rs_ps[:], rs[:], ident[:])
                rs_row = smpool.tile([1, P], F32, tag="rs_row")
                nc.vector.tensor_copy(rs_row[:], rs_ps[:])
                negcorr = smpool.tile([1, B], F32, tag="negcorr")
                nc.vector.tensor_reduce(
                    negcorr[:], rs_row[:].rearrange("r (b g) -> r b g", g=8),
                    axis=mybir.AxisListType.X, op=mybir.AluOpType.add,
                )
                nc.vector.tensor_scalar(
                    negcorr[:], negcorr[:], -OFF, None,
                    op0=mybir.AluOpType.mult,
                )

                # ---- per-block emitters ----
                def act_abs(qb, wb, lo, hi, j, terms):
                    a = apool.tile([P, 1], F32, tag=f"asum{j}",
                                   name=f"asum{j}")
                    if "abs" in skip:
                        nc.vector.memset(a[:], 0.01)
                    else:
                        nc.scalar.activation(
                            qb[:, lo:hi], wb[:, lo:hi],
                            mybir.ActivationFunctionType.Abs,
                            bias=0.0, scale=1.0 / K, accum_out=a[:],
                        )
                    terms.append((a, False))

                def emit_abs_h0(i, wb, tail=False):
                    qb = qpool.tile([P, K], BF16, tag="qb")
                    terms = []
                    act_abs(qb, wb, 0, HALF, 0, terms)
                    return qb, terms

                def emit_abs_h1(i, wb, qb, terms, tail=False):
                    if not tail:
                        act_abs(qb, wb, HALF, K, 1, terms)
                    else:
                        # second half lands as 4 sub-quarter DMAs; chase
                        # them (3 ACT pieces + a DVE tail piece) so alpha
                        # is ready ~2.5us after the last byte
                        act_abs(qb, wb, HALF, 3 * QTR, 1, terms)
                        act_abs(qb, wb, 3 * QTR, 3 * QTR + 1024, 2, terms)
                        cut = 3 * QTR + 1536
                        act_abs(qb, wb, 3 * QTR + 1024, cut, 3, terms)
                        a = apool.tile([P, 1], F32, tag="asum5",
                                       name="asum5")
                        if "abs" in skip:
                            nc.vector.memset(a[:], 0.0)
                        else:
                            nc.vector.tensor_reduce(
                                a[:], wb[:, cut:K],
                                axis=mybir.AxisListType.X,
                                op=mybir.AluOpType.add,
                                apply_absolute_value=True,
                            )
                        terms.append((a, True))

                def emit_alpha(terms):
                    """alpha_sc (= alpha*M/127) and rinv, both [P,1].
                    Only rinv gates the quant; alpha_sc is epilogue-only."""
                    alpha = apool.tile([P, 1], F32, tag="alpha")
                    acc = None
                    for j, (a, raw) in enumerate(terms):
                        t = a
                        if raw:
                            t = apool.tile([P, 1], F32, tag=f"sc{j}",
                                           name=f"sc{j}")
                            nc.vector.tensor_scalar(
                                t[:], a[:], 1.0 / K, None,
                                op0=mybir.AluOpType.mult,
                            )
                        if acc is None:
                            acc = t
                        else:
                            nxt = alpha if j == len(terms) - 1 else \
                                apool.tile([P, 1], F32, tag=f"ac{j}",
                                           name=f"ac{j}")
                            nc.vector.tensor_tensor(
                                nxt[:], acc[:], t[:], op=mybir.AluOpType.add
                            )
                            acc = nxt
                    nc.vector.tensor_scalar(
                        alpha[:], alpha[:], EPS, None, op0=mybir.AluOpType.max
                    )
                    ra = apool.tile([P, 1], F32, tag="ra")
                    nc.vector.reciprocal(ra[:], alpha[:])
                    ea = apool.tile([P, 1], F32, tag="ea")
                    nc.vector.tensor_tensor(
                        ea[:], alpha[:], ra[:], op=mybir.AluOpType.mult
                    )
                    nc.vector.tensor_scalar(
                        ea[:], ea[:], -1.0, 2.0,
                        op0=mybir.AluOpType.mult, op1=mybir.AluOpType.add,
                    )
                    rinv = apool.tile([P, 1], F32, tag="rinv")
                    nc.vector.tensor_tensor(
                        rinv[:], ra[:], ea[:], op=mybir.AluOpType.mult
                    )
                    alpha_sc = apool.tile([P, 1], F32, tag="alpha_sc")
                    nc.vector.tensor_scalar(
                        alpha_sc[:], alpha[:], s128[:, 1:2], None,
                        op0=mybir.AluOpType.mult,
                    )
                    return alpha_sc, rinv

                def process(i, wb, qb, terms):
                    alpha_sc, rinv = emit_alpha(terms)
                    gbuf = gpool.tile([P, KT * P], BF16, tag="gbuf")
                    psg = ps_y.tile([P, B], F32, tag="psg")
                    tailblk = i == NBLK - 1

                    def act_quant(g):
                        # last block only: ACT is free once its abs is
                        # done (no next block), DVE is the tail critical
                        # path -- offload some quant groups
                        return tailblk and g >= 5

                    def emit_mm_group(g):
                        for t in range(8):
                            kt = g * 8 + t
                            g_, f1 = kt // 8, kt % 8
                            nc.tensor.matmul(
                                psg[:],
                                gbuf[:, kt * P:(kt + 1) * P],
                                xi_v[:, f1, g_, :],
                                start=(kt == 0),
                                stop=False,
                            )

                    def emit_mm_corr():
                        # psgT -= 192*sum_k xi[b,k] (outer ones_o): undoes
                        # the +192 quant offset, f32 k=1 matmul
                        nc.tensor.matmul(
                            psg[:], ones_row[:], negcorr[:],
                            start=False, stop=True,
                        )

                    psts = [None] * NG

                    def emit_quant_t(g):
                        lo = g * GW
                        hi = lo + GW
                        if "quant" not in skip:
                            if act_quant(g):
                                nc.scalar.activation(
                                    qb[:, lo:hi], wb[:, lo:hi],
                                    mybir.ActivationFunctionType.Identity,
                                    bias=off192[:], scale=rinv[:],
                                )
                            else:
                                nc.vector.tensor_scalar(
                                    qb[:, lo:hi], wb[:, lo:hi],
                                    rinv[:], OFF,
                                    op0=mybir.AluOpType.mult,
                                    op1=mybir.AluOpType.add,
                                )
                            if _copy_eng(g, tailblk) == "act":
                                # pre-clamp in SBUF (4x-rate bf16). Kept
                                # on DVE: real-HW gpsimd ops proved far
                                # slower than the cost model (PRECLAMP_ENG
                                # knob to re-test)
                                peng = nc.vector if (
                                    tailblk or PRECLAMP_ENG == "dve"
                                ) else nc.gpsimd
                                peng.tensor_scalar(
                                    qb[:, lo:hi], qb[:, lo:hi],
                                    193.0, 191.0,
                                    op0=mybir.AluOpType.min,
                                    op1=mybir.AluOpType.max,
                                )
                        pst = ps_t.tile([P, 8 * P], BF16, tag="pst")
                        psts[g] = pst
                        for t in range(8):
                            nc.tensor.transpose(
                                pst[:, t * P:(t + 1) * P],
                                qb[:, lo + t * P:lo + (t + 1) * P],
                                identb[:],
                            )

                    def emit_copy(g):
                        lo = g * GW
                        hi = lo + GW
                        pst = psts[g]
                        if _copy_eng(g, tailblk) == "act":
                            # f32-bitcast halves the ACT element count.
                            # Value-safe ONLY because pre-clamped: bf16
                            # pairs from {191,192,193} form normal f32s,
                            # and Copy's 1.0*x+0.0 is exact for normals.
                            nc.scalar.copy(
                                gbuf[:, lo:hi].bitcast(F32),
                                pst[:, 0:GW].bitcast(F32),
                            )
                        else:
                            nc.vector.tensor_scalar(
                                gbuf[:, lo:hi], pst[:, 0:GW],
                                193.0, 191.0,
                                op0=mybir.AluOpType.min,
                                op1=mybir.AluOpType.max,
                            )

                    if "mm" in skip:
                        nc.vector.memset(gbuf[:, 0:1], 0.0)
                        nc.tensor.matmul(
                            psg[:], gbuf[:, 0:P], xi_v[:, 0, 0, :],
                            start=True, stop=True,
                        )
                    else:
                        for g in range(NG):
                            emit_quant_t(g)
                            if g >= 1:
                                emit_copy(g - 1)
                            if g >= 2 and not tailblk:
                                emit_mm_group(g - 2)
                        emit_copy(NG - 1)
                        if tailblk:
                            for g in range(NG):
                                emit_mm_group(g)
                        else:
                            emit_mm_group(NG - 2)
                            emit_mm_group(NG - 1)
                        emit_mm_corr()

                    return psg, alpha_sc

                # ---- main loop ----
                # epilogue runs one block late and y stores two blocks
                # late: by then their inputs are long ready, so neither
                # ever head-of-line blocks an in-order engine / the SP
                # weight ring
                def emit_epi(j, psg, alpha_sc):
                    ysb = smpool.tile([P, B], F32, tag="ysb")
                    nc.scalar.activation(
                        ysb[:], psg[:],
                        mybir.ActivationFunctionType.Identity,
                        bias=biasc[:, j:j + 1], scale=alpha_sc[:],
                    )
                    return ysb

                def emit_y_store(j, ysb):
                    nc.sync.dma_start(y_d[j * P:(j + 1) * P, :], ysb[:])

                absd = emit_abs_h0(0, blk_wb[0], tail=(NBLK == 1))
                emit_abs_h1(0, blk_wb[0], *absd, tail=(NBLK == 1))
                pss = [None] * NBLK
                ysbs = [None] * NBLK
                for i in range(NBLK):
                    if i + 2 < NBLK:
                        blk_wb[i + 2] = emit_load(
                            i + 2, tail=(i + 2 == NBLK - 1))
                    if i >= 2:
                        emit_y_store(i - 2, ysbs[i - 2])
                    if i >= 1:
                        ysbs[i - 1] = emit_epi(i - 1, *pss[i - 1])
                    cur = absd
                    if i + 1 < NBLK:
                        absd = emit_abs_h0(i + 1, blk_wb[i + 1],
                                           tail=(i + 1 == NBLK - 1))
                    pss[i] = process(i, blk_wb[i], *cur)
                    if i + 1 < NBLK:
                        emit_abs_h1(i + 1, blk_wb[i + 1], *absd,
                                    tail=(i + 1 == NBLK - 1))
                ysbs[NBLK - 1] = emit_epi(NBLK - 1, *pss[NBLK - 1])
                if NBLK >= 2:
                    emit_y_store(NBLK - 2, ysbs[NBLK - 2])
                emit_y_store(NBLK - 1, ysbs[NBLK - 1])

    nc.finalize()
    return nc


def _get_nc():
    if "nc" not in _CACHE:
        _CACHE["nc"] = _build_bass()
    return _CACHE["nc"]


def _in_maps(x, weight, bias):
    ident = np.eye(P, dtype=np.float32)
    identb = _f32_to_bf16(np.eye(P, dtype=np.float32))
    maps = []
    for c in range(NCORES):
        wsh = np.ascontiguousarray(weight[c * OSH:(c + 1) * OSH])
        bsh = np.ascontiguousarray(
            bias[c * OSH:(c + 1) * OSH].reshape(NBLK, P).T
        )
        maps.append({"x": x, "w": wsh, "biasc": bsh, "ident": ident,
                     "identb": identb})
    return maps


def _f32_to_bf16(a):
    """f32 -> bf16 (RNE) as a uint16-backed ml_dtypes array if available,
    else via jax; run_bass_kernel_spmd wants a real bf16 ndarray."""
    try:
        import ml_dtypes
        return a.astype(ml_dtypes.bfloat16)
    except ImportError:
        import jax.numpy as jnp
        return np.asarray(jnp.asarray(a, dtype=jnp.bfloat16))


def kernel(x, weight, bias):
    x = np.ascontiguousarray(x, dtype=np.float32)
    weight = np.ascontiguousarray(weight, dtype=np.float32)
    bias = np.ascontiguousarray(bias, dtype=np.float32)

    nc = _get_nc()
    os.environ["BASS_NEVER_TRACE"] = "1"
    res = run_bass_kernel_spmd(nc, _in_maps(x, weight, bias),
                               list(range(NCORES)))
    _CACHE["last"] = res
    # y comes back [OSH, B] per core (transposed orientation)
    y = np.concatenate(
        [np.asarray(res.results[c]["y"]).T for c in range(NCORES)], axis=1
    )
    return np.ascontiguousarray(y, dtype=np.float32)


if __name__ == "__main__":
    rng = np.random.default_rng(0)
    x = rng.standard_normal((B, K), dtype=np.float32)
    w = rng.standard_normal((OUT, K), dtype=np.float32) * 0.01
    b = rng.standard_normal(OUT, dtype=np.float32) * 0.01
    y = kernel(x=x, weight=w, bias=b)
    print(y.shape, y.dtype)
